# revision 25
# baseline (speedup 1.0000x reference)
"""CategoryDense (nn_CategoryDense) TRN2 Bass kernel — bf16 I/O version.

out[b, c, o] = sum_i x[b, c, i] * kernel[0, c, i, o] + bias[0, c, o]
x: [8192, 64, 64] f32; kernel: [1, 64, 64, 64]; bias: [1, 64, 64].

Data-parallel over 8 NeuronCores: batch dim sharded 1024 rows/core,
weights + bias replicated; no cross-core communication.

The rel-err gate (2e-2 of output absmax) leaves ~100x headroom over
f32, so all HBM traffic is bf16: the host rounds x and the weight
stacks to bf16, the kernel stores bf16 outputs, and the host upcasts.
Per-core HBM drops 33.6MB -> 17.3MB => ~48us roofline at 358 GB/s.

Per-core kernel (Tile framework), per 128-row b-tile of x ([128, 2048]
bf16), pipelined in groups of 4 category-pairs:
  - 4 "transpose" matmuls (lhsT = x block [128b, 128ci], rhs = bf16
    identity => psT[ci, b]) into one [128, 512] f32 PSUM bank. Regular
    matmuls pipeline at ~81ns vs ~275ns for PE transpose-mode.
  - One ACT copy psT -> SBUF xT as bf16 (rounds exactly; values are
    already bf16).
  - 8 weight matmuls per PAIR of groups (lhsT = xT block, rhs =
    [128, 128] bf16 block-diagonal weight stack for cats 2j/2j+1)
    into one [128, 1024] f32 2-bank PSUM tile.
  - One DVE add of partition-replicated bf16 bias per [128, 1024],
    writing the bf16 out tile (wide adds amortize DVE op overhead;
    DVE is the most-loaded drain engine).
  - W-pair h is emitted a couple of T-groups late so the ACT copy
    latency hides under other groups' PE work.
Engine-budget per b-tile: DMA 5.9us, PE ~5.2us, ACT 8 copies ~5.8us,
DVE 4 adds ~5.2us -> DMA-limited around the bf16 HBM roofline.

Placement notes (from perfetto traces):
  - Every HWDGE dma_start costs ~800ns of DIRECT2D dispatch on the
    issuing sequencer, so x0 is split [512, 1536, 2048] (not quarters)
    and stores ride the SP ring where dispatch doesn't stall ACT
    copies.
  - GPSIMD is kept off the critical path entirely: its SWDGE
    PartitionBroadcast + memset + drains serialized ~14us of startup
    (observed 4.4us PE stall waiting on w_all zero paint). Zeros come
    from a DVE multiply-by-0 instead.
  - Bias is broadcast across partitions with K=1 ones-matmuls + DVE
    copies during the ramp (PE idles on x chunks there anyway); a DMA
    partition_broadcast would push ~1MB through the SDMA fleet exactly
    when the x0/x1 stream is the critical path.
"""

from contextlib import ExitStack

import ml_dtypes
import numpy as np

import concourse.bass as bass  # noqa: F401  (engine namespaces live on nc)
import concourse.mybir as mybir
import concourse.tile as tile
from concourse import bacc
from concourse.bass_utils import run_bass_kernel_spmd


F32 = mybir.dt.float32
BF16 = mybir.dt.bfloat16
NP_BF16 = ml_dtypes.bfloat16

N_CORES = 8
B, C, IN, OUT = 8192, 64, 64, 64
B_SHARD = B // N_CORES
N_PAIRS = C // 2  # category pairs per 128-wide block
GRP = 4  # pairs per T-group / PSUM bank
N_GRP = N_PAIRS // GRP  # 8 T-groups per b-tile
GCOLS = GRP * 128  # 512 = one f32 PSUM bank
WCOLS = 2 * GCOLS  # W-super spans two T-groups (2 PSUM banks)


def _build_nc(b_shard=B_SHARD, lagw=2):
    n_btiles = b_shard // 128
    CI = C * IN
    CO = C * OUT

    nc = bacc.Bacc("TRN2", target_bir_lowering=False, debug=False)
    x = nc.dram_tensor("x", [b_shard, C, IN], BF16, kind="ExternalInput").ap()
    # Host-prepared compact weight stacks (see kernel() below)
    wstack = nc.dram_tensor("wstack", [128, N_PAIRS, OUT], BF16,
                            kind="ExternalInput").ap()
    bias = nc.dram_tensor("bias", [1, C, OUT], BF16,
                      kind="ExternalInput").ap()
    ident_in = nc.dram_tensor("ident", [128, 128], BF16,
                              kind="ExternalInput").ap()
    out = nc.dram_tensor("out", [b_shard, C, OUT], BF16,
                         kind="ExternalOutput").ap()

    x_t = x.rearrange("(t p) c i -> t p (c i)", p=128)
    out_t = out.rearrange("(t p) c o -> t p (c o)", p=128)

    with tile.TileContext(nc) as tc, ExitStack() as ctx:
        const_pool = ctx.enter_context(tc.tile_pool(name="const", bufs=1))
        # 4 x bufs: slow-mode traces show ~3us PE stalls on late x-tile
        # loads when store/load interleaving on the SP ring lines up
        # badly; a deeper prefetch window absorbs that jitter.
        x_pool = ctx.enter_context(tc.tile_pool(name="x", bufs=4))
        out_pool = ctx.enter_context(tc.tile_pool(name="out", bufs=3))
        xt_pool = ctx.enter_context(tc.tile_pool(name="xt", bufs=8))
        # 2 + 3x2 = 8 PSUM banks. ps_o gets 3 bufs: with only 2, the
        # trace shows PE stalling ~1.4us on W-super(s+2) waiting for
        # add(s) to drain a bank. psT turns over quickly (ACT copies
        # start immediately), so 2 bufs suffice there.
        psum_t = ctx.enter_context(
            tc.tile_pool(name="psum_t", bufs=2, space="PSUM"))
        psum_o = ctx.enter_context(
            tc.tile_pool(name="psum_o", bufs=3, space="PSUM"))

        # ACT ring: ident first (first T-matmul needs it), then the bias
        # broadcast halves (first DVE add needs half 0 by ~13us), then
        # the weight stacks (first W-matmul slightly later).
        ident = const_pool.tile([128, 128], BF16)
        nc.scalar.dma_start(ident[:], ident_in[:])

        # SP ring: x0 in [512, 1536, 2048]-column chunks so the first
        # T-group starts ~3.5us earlier than a monolithic 1MB load
        # (each dma_start costs ~800ns of serialized dispatch). SWDGE
        # (gpsimd) is NOT used for any chunk: its descriptor rings
        # contend for the AXI ports serving SDMA engines 7/15 and
        # showed up as hot engines + load-completion tail latency.
        x0_sb = x_pool.tile([128, CI], BF16, tag="x_sb")
        for lo, hi in ((0, 512), (512, 2048), (2048, 4096)):
            nc.sync.dma_start(x0_sb[:, lo:hi], x_t[0][:, lo:hi])
        x1_sb = x_pool.tile([128, CI], BF16, tag="x_sb")

        # Block-diagonal weight stacks built on-chip from the compact
        # 0.5MB load: DVE paints the off-diagonal zeros (broadcast
        # source) and copies the diagonal blocks. Zeros come from a
        # multiply-by-0 on ident (NOT gpsimd memset: Q7 serialization
        # held the zero paint until ~28us in the v1 trace; not wc_sb:
        # its DMA lands late).
        bias_row = const_pool.tile([1, CO], BF16)
        ones_t = const_pool.tile([1, 128], BF16)
        nc.scalar.activation(ones_t[:], ident[0:1, :],
                             mybir.ActivationFunctionType.Copy,
                             bias=1.0, scale=0.0)
        wc_sb = const_pool.tile([128, N_PAIRS, OUT], BF16)
        nc.scalar.dma_start(bias_row[:], bias.rearrange("a c o -> a (c o)"))
        nc.scalar.dma_start(wc_sb[:], wstack[:])
        zero_t = const_pool.tile([128, OUT], BF16)
        nc.vector.tensor_scalar_mul(zero_t[:], ident[:, 0:OUT], 0.0)
        w_all = const_pool.tile([128, N_PAIRS, 128], BF16)
        nc.vector.tensor_copy(
            out=w_all[0:IN, :, OUT:128],
            in_=zero_t[0:IN, None, :].to_broadcast([IN, N_PAIRS, OUT]))
        nc.vector.tensor_copy(
            out=w_all[IN:128, :, 0:OUT],
            in_=zero_t[IN:128, None, :].to_broadcast([IN, N_PAIRS, OUT]))
        nc.vector.tensor_copy(out=w_all[0:IN, :, 0:OUT], in_=wc_sb[0:IN])
        nc.vector.tensor_copy(out=w_all[IN:128, :, OUT:128], in_=wc_sb[IN:128])

        # x1 right behind the x0 chunks on the SP ring (bias dispatches
        # ahead of it cost a ~2us tile-1 stall in the v3 trace).
        nc.sync.dma_start(x1_sb[:], x_t[1])

        # Bias: load only the [1, CO] row (8KB) and broadcast it across
        # partitions with K=1 ones-matmuls + DVE copies, slotted into
        # the ramp where the PE idles waiting for x chunks anyway. The
        # DMA partition_broadcast alternative pushes ~1MB through the
        # SDMA fleet exactly when the x0/x1 stream is the critical path.
        bias_sb = const_pool.tile([128, CO], BF16)

        def emit_bias_quarter(h):
            ps_b = psum_o.tile([128, WCOLS], F32, name="ps_o", tag="ps_o")
            for k in range(2):
                lo = h * WCOLS + k * GCOLS
                nc.tensor.matmul(ps_b[:, k * GCOLS:(k + 1) * GCOLS],
                                 lhsT=ones_t[:], rhs=bias_row[:, lo:lo + GCOLS],
                                 start=True, stop=True)
            nc.vector.tensor_copy(
                out=bias_sb[:, h * WCOLS:(h + 1) * WCOLS], in_=ps_b[:])

        x_sbs = [x0_sb, x1_sb] + [None] * (n_btiles - 2)
        o_sbs = [None] * n_btiles

        def emit_T(t, g, x_sb):
            psT = psum_t.tile([128, GCOLS], F32)
            for qq in range(GRP):
                j = g * GRP + qq
                nc.tensor.matmul(psT[:, qq * 128:(qq + 1) * 128],
                                 lhsT=x_sb[:, j * 128:(j + 1) * 128],
                                 rhs=ident[:], start=True, stop=True)
            xT = xt_pool.tile([128, GCOLS], BF16)
            nc.scalar.copy(xT[:], psT[:])
            return xT

        def emit_W(t, h, xT0, xT1):
            # One W-super: 8 matmuls (pairs 8h..8h+7) into a 2-bank
            # PSUM tile, then a single wide DVE bias-add.
            ps_o = psum_o.tile([128, WCOLS], F32, name="ps_o", tag="ps_o")
            for half_i, xT in enumerate((xT0, xT1)):
                for qq in range(GRP):
                    j = ((2 * h + half_i) % N_GRP) * GRP + qq
                    col = half_i * GCOLS + qq * 128
                    nc.tensor.matmul(ps_o[:, col:col + 128],
                                     lhsT=xT[:, qq * 128:(qq + 1) * 128],
                                     rhs=w_all[:, j], start=True, stop=True)
            hw = h % (N_GRP // 2)
            lo = hw * WCOLS
            if t == n_btiles - 1 and hw == N_GRP // 2 - 1:
                # Final super: 512-wide add/store pairs so the last
                # store starts one DVE-add earlier.
                for k in range(2):
                    sl = lo + k * GCOLS
                    nc.vector.tensor_add(
                        out=o_sbs[t][:, sl:sl + GCOLS],
                        in0=ps_o[:, k * GCOLS:(k + 1) * GCOLS],
                        in1=bias_sb[:, sl:sl + GCOLS])
                    nc.sync.dma_start(out_t[t][:, sl:sl + GCOLS],
                                      o_sbs[t][:, sl:sl + GCOLS])
                return
            nc.vector.tensor_add(
                out=o_sbs[t][:, lo:lo + WCOLS],
                in0=ps_o[:], in1=bias_sb[:, lo:lo + WCOLS])
            # SP-ring stores: dispatch doesn't stall ACT copies.
            if t == n_btiles - 1:
                # Last tile drains per-super so the tail overlaps.
                nc.sync.dma_start(out_t[t][:, lo:lo + WCOLS],
                                  o_sbs[t][:, lo:lo + WCOLS])
            elif hw == N_GRP // 2 - 1:
                nc.sync.dma_start(out_t[t], o_sbs[t][:])

        total = n_btiles * N_GRP
        pend = {}
        for G in range(total + lagw + 2):
            if G < total:
                t, g = divmod(G, N_GRP)
                if g == 0:
                    if t + 2 < n_btiles:
                        xs = x_pool.tile([128, CI], BF16, tag="x_sb")
                        nc.sync.dma_start(xs[:], x_t[t + 2])
                        x_sbs[t + 2] = xs
                    o_sbs[t] = out_pool.tile([128, CO], BF16, name="o_sb",
                                             tag="o_sb")
                pend[G] = emit_T(t, g, x_sbs[t])
                if t == 0 and 1 <= g <= N_GRP // 2:
                    # Bias quarter g-1 is ready before add(g-1); the
                    # ones-matmuls execute in the PE's ramp idle gaps.
                    emit_bias_quarter(g - 1)
            Gr = G - lagw
            if Gr >= 1 and Gr % 2 == 1 and (Gr in pend):
                h = Gr // 2
                tw = (2 * h) // N_GRP
                emit_W(tw, h, pend.pop(Gr - 1), pend.pop(Gr))

    nc.compile()
    return nc


_NC_CACHE = {}


def _get_nc():
    if "nc" not in _NC_CACHE:
        _NC_CACHE["nc"] = _build_nc()
    return _NC_CACHE["nc"]


def _install_ntff_shim():
    """Profiling only: register the axon NTFF hook under antenv.axon_hooks.

    The container's antenv stub lacks axon_hooks, so bass_utils'
    `from antenv.axon_hooks import get_axon_ntff_profile_hook` raises on
    trace=True runs. Recreate the module from trn_agent_boot's ctypes hook.
    """
    import sys
    import types

    if "antenv.axon_hooks" in sys.modules:
        return
    from trn_agent_boot.trn_boot import _ntff_profile_via_ctypes

    hook = _ntff_profile_via_ctypes("/opt/axon/libaxon_pjrt.so")
    mod = types.ModuleType("antenv.axon_hooks")
    mod.get_axon_ntff_profile_hook = lambda: hook
    mod.set_axon_ntff_profile_hook = lambda h: None
    sys.modules["antenv.axon_hooks"] = mod
    import antenv

    antenv.axon_hooks = mod


def kernel(x, kernel, bias, _trace=False, _trace_kwargs=None):
    x = np.ascontiguousarray(x, dtype=np.float32)
    kernel = np.ascontiguousarray(kernel, dtype=np.float32)
    bias = np.ascontiguousarray(bias, dtype=np.float32)
    assert x.shape == (B, C, IN)

    if _trace:
        _install_ntff_shim()
    nc = _get_nc()
    x_bf = x.astype(NP_BF16)
    # Compact weight stacks: wstack[p, j, :] holds cat 2j's [i, o] block
    # for p < 64 and cat 2j+1's for p >= 64 (block-diag built on-chip).
    wstack = np.empty((128, N_PAIRS, OUT), dtype=np.float32)
    wstack[0:IN] = kernel[0, 0::2].transpose(1, 0, 2)
    wstack[IN:128] = kernel[0, 1::2].transpose(1, 0, 2)
    wstack = wstack.astype(NP_BF16)
    ident = np.eye(128, dtype=np.float32).astype(NP_BF16)
    in_maps = [
        {
            "x": x_bf[i * B_SHARD:(i + 1) * B_SHARD],
            "wstack": wstack,
            "bias": bias.astype(NP_BF16),
            "ident": ident,
        }
        for i in range(N_CORES)
    ]
    res = run_bass_kernel_spmd(
        nc, in_maps, core_ids=list(range(N_CORES)),
        trace=_trace, **(_trace_kwargs or {})
    )
    out = np.concatenate(
        [res.results[i]["out"] for i in range(N_CORES)], axis=0
    ).astype(np.float32)
    if _trace:
        _NC_CACHE["last_results"] = res
    return out


# revision 26
# speedup vs baseline: 1.0102x; 1.0102x over previous
"""CategoryDense (nn_CategoryDense) TRN2 Bass kernel — bf16 I/O version.

out[b, c, o] = sum_i x[b, c, i] * kernel[0, c, i, o] + bias[0, c, o]
x: [8192, 64, 64] f32; kernel: [1, 64, 64, 64]; bias: [1, 64, 64].

Data-parallel over 8 NeuronCores: batch dim sharded 1024 rows/core,
weights + bias replicated; no cross-core communication.

The rel-err gate (2e-2 of output absmax) leaves ~100x headroom over
f32, so all HBM traffic is bf16: the host rounds x and the weight
stacks to bf16, the kernel stores bf16 outputs, and the host upcasts.
Per-core HBM drops 33.6MB -> 17.3MB => ~48us roofline at 358 GB/s.

Per-core kernel (Tile framework), per 128-row b-tile of x ([128, 2048]
bf16), pipelined in groups of 4 category-pairs:
  - 4 "transpose" matmuls (lhsT = x block [128b, 128ci], rhs = bf16
    identity => psT[ci, b]) into one [128, 512] f32 PSUM bank. Regular
    matmuls pipeline at ~81ns vs ~275ns for PE transpose-mode.
  - One ACT copy psT -> SBUF xT as bf16 (rounds exactly; values are
    already bf16).
  - 8 weight matmuls per PAIR of groups (lhsT = xT block, rhs =
    [128, 128] bf16 block-diagonal weight stack for cats 2j/2j+1)
    into one [128, 1024] f32 2-bank PSUM tile.
  - One DVE add of partition-replicated bf16 bias per [128, 1024],
    writing the bf16 out tile (wide adds amortize DVE op overhead;
    DVE is the most-loaded drain engine).
  - W-pair h is emitted a couple of T-groups late so the ACT copy
    latency hides under other groups' PE work.
Engine-budget per b-tile: DMA 5.9us, PE ~5.2us, ACT 8 copies ~5.8us,
DVE 4 adds ~5.2us -> DMA-limited around the bf16 HBM roofline.

Placement notes (from perfetto traces):
  - Every HWDGE dma_start costs ~800ns of DIRECT2D dispatch on the
    issuing sequencer, so x0 is split [512, 1536, 2048] (not quarters)
    and stores ride the SP ring where dispatch doesn't stall ACT
    copies.
  - GPSIMD is kept off the critical path entirely: its SWDGE
    PartitionBroadcast + memset + drains serialized ~14us of startup
    (observed 4.4us PE stall waiting on w_all zero paint). Zeros come
    from a DVE multiply-by-0 instead.
  - Bias is a bf16 HWDGE partition_broadcast from DRAM (a PE
    ones-matmul broadcast measured ~5us worse: cold-PE matmuls + PSUM
    pool pressure during the ramp outweigh the ~1MB DMA saving).
"""

from contextlib import ExitStack

import ml_dtypes
import numpy as np

import concourse.bass as bass  # noqa: F401  (engine namespaces live on nc)
import concourse.mybir as mybir
import concourse.tile as tile
from concourse import bacc
from concourse.bass_utils import run_bass_kernel_spmd


F32 = mybir.dt.float32
BF16 = mybir.dt.bfloat16
NP_BF16 = ml_dtypes.bfloat16

N_CORES = 8
B, C, IN, OUT = 8192, 64, 64, 64
B_SHARD = B // N_CORES
N_PAIRS = C // 2  # category pairs per 128-wide block
GRP = 4  # pairs per T-group / PSUM bank
N_GRP = N_PAIRS // GRP  # 8 T-groups per b-tile
GCOLS = GRP * 128  # 512 = one f32 PSUM bank
WCOLS = 2 * GCOLS  # W-super spans two T-groups (2 PSUM banks)


def _build_nc(b_shard=B_SHARD, lagw=2):
    n_btiles = b_shard // 128
    CI = C * IN
    CO = C * OUT

    nc = bacc.Bacc("TRN2", target_bir_lowering=False, debug=False)
    x = nc.dram_tensor("x", [b_shard, C, IN], BF16, kind="ExternalInput").ap()
    # Host-prepared compact weight stacks (see kernel() below)
    wstack = nc.dram_tensor("wstack", [128, N_PAIRS, OUT], BF16,
                            kind="ExternalInput").ap()
    bias = nc.dram_tensor("bias", [1, C, OUT], BF16,
                      kind="ExternalInput").ap()
    ident_in = nc.dram_tensor("ident", [128, 128], BF16,
                              kind="ExternalInput").ap()
    out = nc.dram_tensor("out", [b_shard, C, OUT], BF16,
                         kind="ExternalOutput").ap()

    x_t = x.rearrange("(t p) c i -> t p (c i)", p=128)
    out_t = out.rearrange("(t p) c o -> t p (c o)", p=128)

    with tile.TileContext(nc) as tc, ExitStack() as ctx:
        const_pool = ctx.enter_context(tc.tile_pool(name="const", bufs=1))
        # 4 x bufs: slow-mode traces show ~3us PE stalls on late x-tile
        # loads when store/load interleaving on the SP ring lines up
        # badly; a deeper prefetch window absorbs that jitter.
        x_pool = ctx.enter_context(tc.tile_pool(name="x", bufs=4))
        out_pool = ctx.enter_context(tc.tile_pool(name="out", bufs=3))
        xt_pool = ctx.enter_context(tc.tile_pool(name="xt", bufs=8))
        # 2 + 3x2 = 8 PSUM banks. ps_o gets 3 bufs: with only 2, the
        # trace shows PE stalling ~1.4us on W-super(s+2) waiting for
        # add(s) to drain a bank. psT turns over quickly (ACT copies
        # start immediately), so 2 bufs suffice there.
        psum_t = ctx.enter_context(
            tc.tile_pool(name="psum_t", bufs=2, space="PSUM"))
        psum_o = ctx.enter_context(
            tc.tile_pool(name="psum_o", bufs=3, space="PSUM"))

        # ACT ring: ident first (first T-matmul needs it), then the bias
        # broadcast halves (first DVE add needs half 0 by ~13us), then
        # the weight stacks (first W-matmul slightly later).
        ident = const_pool.tile([128, 128], BF16)
        nc.scalar.dma_start(ident[:], ident_in[:])

        # SP ring: x0 in [512, 1536, 2048]-column chunks so the first
        # T-group starts ~3.5us earlier than a monolithic 1MB load
        # (each dma_start costs ~800ns of serialized dispatch). SWDGE
        # (gpsimd) is NOT used for any chunk: its descriptor rings
        # contend for the AXI ports serving SDMA engines 7/15 and
        # showed up as hot engines + load-completion tail latency.
        x0_sb = x_pool.tile([128, CI], BF16, tag="x_sb")
        for lo, hi in ((0, 512), (512, 2048), (2048, 4096)):
            nc.sync.dma_start(x0_sb[:, lo:hi], x_t[0][:, lo:hi])
        x1_sb = x_pool.tile([128, CI], BF16, tag="x_sb")

        # Block-diagonal weight stacks built on-chip from the compact
        # 0.5MB load: DVE paints the off-diagonal zeros (broadcast
        # source) and copies the diagonal blocks. Zeros come from a
        # multiply-by-0 on ident (NOT gpsimd memset: Q7 serialization
        # held the zero paint until ~28us in the v1 trace; not wc_sb:
        # its DMA lands late).
        wc_sb = const_pool.tile([128, N_PAIRS, OUT], BF16)
        nc.scalar.dma_start(wc_sb[:], wstack[:])
        zero_t = const_pool.tile([128, OUT], BF16)
        nc.vector.tensor_scalar_mul(zero_t[:], ident[:, 0:OUT], 0.0)
        w_all = const_pool.tile([128, N_PAIRS, 128], BF16)
        nc.vector.tensor_copy(
            out=w_all[0:IN, :, OUT:128],
            in_=zero_t[0:IN, None, :].to_broadcast([IN, N_PAIRS, OUT]))
        nc.vector.tensor_copy(
            out=w_all[IN:128, :, 0:OUT],
            in_=zero_t[IN:128, None, :].to_broadcast([IN, N_PAIRS, OUT]))
        nc.vector.tensor_copy(out=w_all[0:IN, :, 0:OUT], in_=wc_sb[0:IN])
        nc.vector.tensor_copy(out=w_all[IN:128, :, OUT:128], in_=wc_sb[IN:128])

        # x1 right behind the x0 chunks on the SP ring (bias dispatches
        # ahead of it cost a ~2us tile-1 stall in the v3 trace).
        nc.sync.dma_start(x1_sb[:], x_t[1])

        # Bias on the ACT ring after the weight stacks: first W-matmul
        # needs w_all ~12us, the first DVE add needs bias half 0 a beat
        # later. Halved so half 0 lands sooner. (A K=1 ones-matmul
        # broadcast that skips the 1MB DMA was tried and measured ~5us
        # WORSE: cold-PE bias matmuls + psum_o pool pressure during the
        # ramp outweigh the DMA saving.)
        bias_flat = bias.rearrange("a c o -> a (c o)")
        bias_sb = const_pool.tile([128, CO], BF16)
        half = CO // 2
        nc.scalar.dma_start(bias_sb[:, 0:half],
                            bias_flat[:, 0:half].partition_broadcast(128))
        nc.scalar.dma_start(bias_sb[:, half:CO],
                            bias_flat[:, half:CO].partition_broadcast(128))

        x_sbs = [x0_sb, x1_sb] + [None] * (n_btiles - 2)
        o_sbs = [None] * n_btiles

        def emit_T(t, g, x_sb):
            psT = psum_t.tile([128, GCOLS], F32)
            for qq in range(GRP):
                j = g * GRP + qq
                nc.tensor.matmul(psT[:, qq * 128:(qq + 1) * 128],
                                 lhsT=x_sb[:, j * 128:(j + 1) * 128],
                                 rhs=ident[:], start=True, stop=True)
            xT = xt_pool.tile([128, GCOLS], BF16)
            nc.scalar.copy(xT[:], psT[:])
            return xT

        def emit_W(t, h, xT0, xT1):
            # One W-super: 8 matmuls (pairs 8h..8h+7) into a 2-bank
            # PSUM tile, then a single wide DVE bias-add.
            ps_o = psum_o.tile([128, WCOLS], F32, name="ps_o", tag="ps_o")
            for half_i, xT in enumerate((xT0, xT1)):
                for qq in range(GRP):
                    j = ((2 * h + half_i) % N_GRP) * GRP + qq
                    col = half_i * GCOLS + qq * 128
                    nc.tensor.matmul(ps_o[:, col:col + 128],
                                     lhsT=xT[:, qq * 128:(qq + 1) * 128],
                                     rhs=w_all[:, j], start=True, stop=True)
            hw = h % (N_GRP // 2)
            lo = hw * WCOLS
            if t == n_btiles - 1 and hw == N_GRP // 2 - 1:
                # Final super: 512-wide add/store pairs so the last
                # store starts one DVE-add earlier.
                for k in range(2):
                    sl = lo + k * GCOLS
                    nc.vector.tensor_add(
                        out=o_sbs[t][:, sl:sl + GCOLS],
                        in0=ps_o[:, k * GCOLS:(k + 1) * GCOLS],
                        in1=bias_sb[:, sl:sl + GCOLS])
                    nc.sync.dma_start(out_t[t][:, sl:sl + GCOLS],
                                      o_sbs[t][:, sl:sl + GCOLS])
                return
            nc.vector.tensor_add(
                out=o_sbs[t][:, lo:lo + WCOLS],
                in0=ps_o[:], in1=bias_sb[:, lo:lo + WCOLS])
            # SP-ring stores: dispatch doesn't stall ACT copies.
            if t == n_btiles - 1:
                # Last tile drains per-super so the tail overlaps.
                nc.sync.dma_start(out_t[t][:, lo:lo + WCOLS],
                                  o_sbs[t][:, lo:lo + WCOLS])
            elif hw == N_GRP // 2 - 1:
                nc.sync.dma_start(out_t[t], o_sbs[t][:])

        total = n_btiles * N_GRP
        pend = {}
        for G in range(total + lagw + 2):
            if G < total:
                t, g = divmod(G, N_GRP)
                if g == 0:
                    if t + 2 < n_btiles:
                        xs = x_pool.tile([128, CI], BF16, tag="x_sb")
                        nc.sync.dma_start(xs[:], x_t[t + 2])
                        x_sbs[t + 2] = xs
                    o_sbs[t] = out_pool.tile([128, CO], BF16, name="o_sb",
                                             tag="o_sb")
                pend[G] = emit_T(t, g, x_sbs[t])
            Gr = G - lagw
            if Gr >= 1 and Gr % 2 == 1 and (Gr in pend):
                h = Gr // 2
                tw = (2 * h) // N_GRP
                emit_W(tw, h, pend.pop(Gr - 1), pend.pop(Gr))

    nc.compile()
    return nc


_NC_CACHE = {}


def _get_nc():
    if "nc" not in _NC_CACHE:
        _NC_CACHE["nc"] = _build_nc()
    return _NC_CACHE["nc"]


def _install_ntff_shim():
    """Profiling only: register the axon NTFF hook under antenv.axon_hooks.

    The container's antenv stub lacks axon_hooks, so bass_utils'
    `from antenv.axon_hooks import get_axon_ntff_profile_hook` raises on
    trace=True runs. Recreate the module from trn_agent_boot's ctypes hook.
    """
    import sys
    import types

    if "antenv.axon_hooks" in sys.modules:
        return
    from trn_agent_boot.trn_boot import _ntff_profile_via_ctypes

    hook = _ntff_profile_via_ctypes("/opt/axon/libaxon_pjrt.so")
    mod = types.ModuleType("antenv.axon_hooks")
    mod.get_axon_ntff_profile_hook = lambda: hook
    mod.set_axon_ntff_profile_hook = lambda h: None
    sys.modules["antenv.axon_hooks"] = mod
    import antenv

    antenv.axon_hooks = mod


def kernel(x, kernel, bias, _trace=False, _trace_kwargs=None):
    x = np.ascontiguousarray(x, dtype=np.float32)
    kernel = np.ascontiguousarray(kernel, dtype=np.float32)
    bias = np.ascontiguousarray(bias, dtype=np.float32)
    assert x.shape == (B, C, IN)

    if _trace:
        _install_ntff_shim()
    nc = _get_nc()
    x_bf = x.astype(NP_BF16)
    # Compact weight stacks: wstack[p, j, :] holds cat 2j's [i, o] block
    # for p < 64 and cat 2j+1's for p >= 64 (block-diag built on-chip).
    wstack = np.empty((128, N_PAIRS, OUT), dtype=np.float32)
    wstack[0:IN] = kernel[0, 0::2].transpose(1, 0, 2)
    wstack[IN:128] = kernel[0, 1::2].transpose(1, 0, 2)
    wstack = wstack.astype(NP_BF16)
    ident = np.eye(128, dtype=np.float32).astype(NP_BF16)
    in_maps = [
        {
            "x": x_bf[i * B_SHARD:(i + 1) * B_SHARD],
            "wstack": wstack,
            "bias": bias.astype(NP_BF16),
            "ident": ident,
        }
        for i in range(N_CORES)
    ]
    res = run_bass_kernel_spmd(
        nc, in_maps, core_ids=list(range(N_CORES)),
        trace=_trace, **(_trace_kwargs or {})
    )
    out = np.concatenate(
        [res.results[i]["out"] for i in range(N_CORES)], axis=0
    ).astype(np.float32)
    if _trace:
        _NC_CACHE["last_results"] = res
    return out
